# revision 20
# baseline (speedup 1.0000x reference)
"""Trainium2 Bass kernel for a small MLP: [N,2] -> 32 -> (8x 32) -> 1.

Strategy (data-parallel over 8 cores, batch-sharded):
  - Per core R=262144 rows, processed in 32 supertiles of 8192 rows.
  - A supertile lives in SBUF as [128 partitions, 2048 free]: 4 partition
    blocks (32 hidden channels each) x 4 free blocks (512 rows each) = 16
    groups of 512 batch rows. Group (i,f) = rows s*8192+(4i+f)*512+[0,512).
  - Each layer = 4 matmuls of [K,128]x[K,512] with BLOCK-DIAGONAL weights:
    one instruction advances 4 groups (2048 batch rows) in 512 moving rows.
    float32r end-to-end (DRAM/SBUF tiles typed f32r; drains round to f32r)
    for the 1 cycle/row PE fast path.
  - bias+ReLU drain PSUM->SBUF split across ACT / DVE / Pool by rate.
  - Output layer: Wout replicated x32 -> every PSUM partition holds y;
    copy-drain, DMA partitions {0,32,64,96} to DRAM; bout added on host.
"""

import numpy as np

N = 2097152
H = 32
L = 8
N_CORES = 8
R = N // N_CORES          # 262144 rows per core
FB = 512                  # rows per group
ST_ROWS = 16 * FB         # 8192 rows per supertile
N_ST = R // ST_ROWS       # 32 supertiles per core

# Drain split (free columns of the 2048-wide supertile per engine),
# proportional to engine rates ACT 1.2 / DVE 0.96 cols/ns. GPSIMD (Pool)
# cannot access PSUM on TRN2, so drains are ACT+DVE only.
ACT_COLS = 1136
DVE_COLS = 2048 - ACT_COLS

_CACHE = {}


def _build_nc(n_st=N_ST):
    import concourse.tile as tile
    from concourse import bacc, mybir

    f32 = mybir.dt.float32
    f32r = mybir.dt.float32r

    nc = bacc.Bacc(None, target_bir_lowering=False)
    xt_d = nc.dram_tensor("xt", [8, n_st, 2048], f32r, kind="ExternalInput")
    wm_d = nc.dram_tensor("wmat", [128, 1280], f32r, kind="ExternalInput")
    wb_d = nc.dram_tensor("wbias", [128, 9], f32, kind="ExternalInput")
    out_d = nc.dram_tensor("out", [n_st, 4, 2048], f32, kind="ExternalOutput")

    relu = mybir.ActivationFunctionType.Relu
    alu_add = mybir.AluOpType.add
    alu_max = mybir.AluOpType.max

    with tile.TileContext(nc) as tc:
        with tc.tile_pool(name="wpool", bufs=1) as wpool, \
             tc.tile_pool(name="xpool", bufs=4) as xpool, \
             tc.tile_pool(name="hpool", bufs=4) as hpool, \
             tc.tile_pool(name="pspool", bufs=2, space="PSUM") as pspool:
            w = wpool.tile([128, 1280], f32r)
            nc.sync.dma_start(out=w[:], in_=wm_d[:, :])
            wb = wpool.tile([128, 9], f32)
            nc.sync.dma_start(out=wb[:], in_=wb_d[:, :])

            # Each layer's activation is drained into TWO separate SBUF
            # tiles (cols 0:1024 by ACT, 1024:2048 by DVE). With a single
            # shared tile the framework serializes the two drains (WAW at
            # tile granularity): mm->ACT(1.2us)->DVE(1.2us) was the
            # critical chain stalling the PE ~660ns per group.
            def layer(s, l, h01, h23, kdim):
                ps = pspool.tile([128, 2048], f32)
                wcol = w[0:kdim, 128 * l:128 * (l + 1)]
                nc.tensor.matmul(ps[:, 0:512], wcol, h01[0:kdim, 0:512])
                nc.tensor.matmul(ps[:, 512:1024], wcol, h01[0:kdim, 512:1024])
                if l == 9:
                    ho01 = hpool.tile([128, 1024], f32)
                    nc.scalar.copy(ho01[:], ps[:, 0:1024])
                    nc.tensor.matmul(ps[:, 1024:1536], wcol,
                                     h23[0:kdim, 0:512])
                    nc.tensor.matmul(ps[:, 1536:2048], wcol,
                                     h23[0:kdim, 512:1024])
                    ho23 = hpool.tile([128, 1024], f32)
                    nc.vector.tensor_scalar_add(ho23[:], ps[:, 1024:2048],
                                                0.0)
                    s01 = ho01[:].rearrange("(i r) f -> i r f", r=32)[:, 0, :]
                    nc.sync.dma_start(out=out_d[s, :, 0:1024], in_=s01)
                    s23 = ho23[:].rearrange("(i r) f -> i r f", r=32)[:, 0, :]
                    nc.sync.dma_start(out=out_d[s, :, 1024:2048], in_=s23)
                    return None, None
                bias = wb[:, l:l + 1]
                hn01 = hpool.tile([128, 1024], f32r)
                nc.scalar.activation(hn01[:], ps[:, 0:1024], relu, bias=bias)
                nc.tensor.matmul(ps[:, 1024:1536], wcol, h23[0:kdim, 0:512])
                nc.tensor.matmul(ps[:, 1536:2048], wcol,
                                 h23[0:kdim, 512:1024])
                hn23 = hpool.tile([128, 1024], f32r)
                nc.vector.tensor_scalar(hn23[:], ps[:, 1024:2048],
                                        bias, 0.0, alu_add, alu_max)
                return hn01, hn23

            # Four supertiles in flight: while supertile A's layer-l drain
            # runs on ACT/DVE, the PE does B/C/D's layer-l matmuls. Removes
            # per-layer PE stalls and keeps the PE continuously busy
            # (p-state ramp needs 3us of uninterrupted execution).
            for t in range(n_st // 4):
                ss = [4 * t + j for j in range(4)]
                hs = []
                for s in ss:
                    x01 = xpool.tile([8, 1024], f32r)
                    nc.sync.dma_start(out=x01[:], in_=xt_d[:, s, 0:1024])
                    x23 = xpool.tile([8, 1024], f32r)
                    nc.sync.dma_start(out=x23[:], in_=xt_d[:, s, 1024:2048])
                    hs.append((x01, x23))
                kdim = 8
                for l in range(10):
                    hs = [layer(ss[j], l, hs[j][0], hs[j][1], kdim)
                          for j in range(4)]
                    kdim = 128
    nc.finalize()
    return nc


def _prep_core_inputs(x_shard, wmat, wbias):
    # xt[2i+c, s, 512f+r] = x_shard[s*8192 + (4i+f)*512 + r, c]
    xs = np.ascontiguousarray(x_shard, dtype=np.float32).reshape(
        N_ST, 4, 4, FB, 2)
    xt = np.ascontiguousarray(xs.transpose(1, 4, 0, 2, 3)).reshape(
        8, N_ST, 2048)
    return {"xt": xt, "wmat": wmat, "wbias": wbias}


def _pack_weights(W0, b0, Wh, bh, Wout):
    # Block-diagonal lhsT per layer, 128 cols each:
    #   l=0:    wmat[2i+c, 32i+m]       = W0[m, c]        (K=8 rows used)
    #   l=1..8: wmat[32i+k, 128l+32i+m] = Wh[l-1][m, k]
    #   l=9:    wmat[32i+k, 1152+32i+m] = Wout[0, k]      (replicated x32)
    wmat = np.zeros((128, 1280), dtype=np.float32)
    wbias = np.zeros((128, 9), dtype=np.float32)
    for i in range(4):
        wmat[2 * i:2 * i + 2, 32 * i:32 * i + 32] = W0.T
        for hl in range(L):
            wmat[32 * i:32 * i + 32,
                 128 * (hl + 1) + 32 * i:128 * (hl + 1) + 32 * i + 32] = \
                Wh[hl].T
        wmat[32 * i:32 * i + 32, 1152 + 32 * i:1152 + 32 * i + 32] = \
            Wout[0, :, None]
        wbias[32 * i:32 * i + 32, 0] = b0
        for hl in range(L):
            wbias[32 * i:32 * i + 32, 1 + hl] = bh[hl]
    return wmat, wbias


def kernel(x, W0, b0, Wh, bh, Wout, bout):
    from concourse import bass_utils

    if "nc" not in _CACHE:
        _CACHE["nc"] = _build_nc()
    nc = _CACHE["nc"]

    wmat, wbias = _pack_weights(np.asarray(W0, np.float32),
                                np.asarray(b0, np.float32),
                                np.asarray(Wh, np.float32),
                                np.asarray(bh, np.float32),
                                np.asarray(Wout, np.float32))
    x = np.asarray(x, np.float32)
    in_maps = [_prep_core_inputs(x[c * R:(c + 1) * R], wmat, wbias)
               for c in range(N_CORES)]

    res = bass_utils.run_bass_kernel_spmd(nc, in_maps, list(range(N_CORES)))
    out = np.concatenate([r["out"].reshape(R) for r in res.results])
    return (out.reshape(N, 1) + np.float32(bout[0])).astype(np.float32)


# revision 21
# speedup vs baseline: 1.2686x; 1.2686x over previous
"""Trainium2 Bass kernel for a small MLP: [N,2] -> 32 -> (8x 32) -> 1.

Strategy (data-parallel over 8 cores, batch-sharded):
  - Per core R=262144 rows, processed in 32 supertiles of 8192 rows.
  - A supertile lives in SBUF as [128 partitions, 2048 free]: 4 partition
    blocks (32 hidden channels each) x 4 free blocks (512 rows each) = 16
    groups of 512 batch rows. Group (i,f) = rows s*8192+(4i+f)*512+[0,512).
  - Each layer = 4 matmuls of [K,128]x[K,512] with BLOCK-DIAGONAL weights:
    one instruction advances 4 groups (2048 batch rows) in 512 moving rows.
    float32r end-to-end (DRAM/SBUF tiles typed f32r; drains round to f32r)
    for the 1 cycle/row PE fast path.
  - bias+ReLU drain PSUM->SBUF split across ACT / DVE / Pool by rate.
  - Output layer: Wout replicated x32 -> every PSUM partition holds y;
    copy-drain, DMA partitions {0,32,64,96} to DRAM; bout added on host.
"""

import numpy as np

N = 2097152
H = 32
L = 8
N_CORES = 8
R = N // N_CORES          # 262144 rows per core
FB = 512                  # rows per group
ST_ROWS = 16 * FB         # 8192 rows per supertile
N_ST = R // ST_ROWS       # 32 supertiles per core

# Drain split (free columns of the 2048-wide supertile per engine),
# proportional to engine rates ACT 1.2 / DVE 0.96 cols/ns. GPSIMD (Pool)
# cannot access PSUM on TRN2, so drains are ACT+DVE only.
ACT_COLS = 1136
DVE_COLS = 2048 - ACT_COLS

_CACHE = {}


def _build_nc(n_st=N_ST):
    import concourse.tile as tile
    from concourse import bacc, mybir

    f32 = mybir.dt.float32
    f32r = mybir.dt.float32r

    nc = bacc.Bacc(None, target_bir_lowering=False)
    xt_d = nc.dram_tensor("xt", [8, n_st, 2048], f32r, kind="ExternalInput")
    wm_d = nc.dram_tensor("wmat", [128, 1280], f32r, kind="ExternalInput")
    wb_d = nc.dram_tensor("wbias", [128, 9], f32, kind="ExternalInput")
    out_d = nc.dram_tensor("out", [n_st, 4, 2048], f32, kind="ExternalOutput")

    relu = mybir.ActivationFunctionType.Relu
    alu_add = mybir.AluOpType.add
    alu_max = mybir.AluOpType.max

    with tile.TileContext(nc) as tc:
        with tc.tile_pool(name="wpool", bufs=1) as wpool, \
             tc.tile_pool(name="xpool", bufs=4) as xpool, \
             tc.tile_pool(name="hpool", bufs=4) as hpool, \
             tc.tile_pool(name="pspool", bufs=2, space="PSUM") as pspool:
            w = wpool.tile([128, 1280], f32r)
            nc.sync.dma_start(out=w[:], in_=wm_d[:, :])
            wb = wpool.tile([128, 9], f32)
            nc.sync.dma_start(out=wb[:], in_=wb_d[:, :])

            # Each layer's activation is drained into TWO separate SBUF
            # tiles (cols 0:1024 by ACT, 1024:2048 by DVE). With a single
            # shared tile the framework serializes the two drains (WAW at
            # tile granularity): mm->ACT(1.2us)->DVE(1.2us) was the
            # critical chain stalling the PE ~660ns per group.
            def layer(s, l, h01, h23, kdim):
                ps = pspool.tile([128, 2048], f32)
                wcol = w[0:kdim, 128 * l:128 * (l + 1)]
                nc.tensor.matmul(ps[:, 0:512], wcol, h01[0:kdim, 0:512])
                nc.tensor.matmul(ps[:, 512:1024], wcol, h01[0:kdim, 512:1024])
                nc.tensor.matmul(ps[:, 1024:1536], wcol, h23[0:kdim, 0:512])
                nc.tensor.matmul(ps[:, 1536:2048], wcol,
                                 h23[0:kdim, 512:1024])
                if l == 9:
                    ho01 = hpool.tile([128, 1024], f32)
                    nc.scalar.copy(ho01[:], ps[:, 0:1024])
                    ho23 = hpool.tile([128, 1024], f32)
                    nc.vector.tensor_scalar_add(ho23[:], ps[:, 1024:2048],
                                                0.0)
                    s01 = ho01[:].rearrange("(i r) f -> i r f", r=32)[:, 0, :]
                    nc.sync.dma_start(out=out_d[s, :, 0:1024], in_=s01)
                    s23 = ho23[:].rearrange("(i r) f -> i r f", r=32)[:, 0, :]
                    nc.sync.dma_start(out=out_d[s, :, 1024:2048], in_=s23)
                    return None, None
                bias = wb[:, l:l + 1]
                hn01 = hpool.tile([128, 1024], f32r)
                nc.scalar.activation(hn01[:], ps[:, 0:1024], relu, bias=bias)
                hn23 = hpool.tile([128, 1024], f32r)
                nc.vector.tensor_scalar(hn23[:], ps[:, 1024:2048],
                                        bias, 0.0, alu_add, alu_max)
                return hn01, hn23

            # Four supertiles in flight: while supertile A's layer-l drain
            # runs on ACT/DVE, the PE does B/C/D's layer-l matmuls. Removes
            # per-layer PE stalls and keeps the PE continuously busy
            # (p-state ramp needs 3us of uninterrupted execution).
            for t in range(n_st // 4):
                ss = [4 * t + j for j in range(4)]
                hs = []
                for s in ss:
                    x01 = xpool.tile([8, 1024], f32r)
                    nc.sync.dma_start(out=x01[:], in_=xt_d[:, s, 0:1024])
                    x23 = xpool.tile([8, 1024], f32r)
                    nc.sync.dma_start(out=x23[:], in_=xt_d[:, s, 1024:2048])
                    hs.append((x01, x23))
                kdim = 8
                for l in range(10):
                    hs = [layer(ss[j], l, hs[j][0], hs[j][1], kdim)
                          for j in range(4)]
                    kdim = 128
    nc.finalize()
    return nc


def _prep_core_inputs(x_shard, wmat, wbias):
    # xt[2i+c, s, 512f+r] = x_shard[s*8192 + (4i+f)*512 + r, c]
    xs = np.ascontiguousarray(x_shard, dtype=np.float32).reshape(
        N_ST, 4, 4, FB, 2)
    xt = np.ascontiguousarray(xs.transpose(1, 4, 0, 2, 3)).reshape(
        8, N_ST, 2048)
    return {"xt": xt, "wmat": wmat, "wbias": wbias}


def _pack_weights(W0, b0, Wh, bh, Wout):
    # Block-diagonal lhsT per layer, 128 cols each:
    #   l=0:    wmat[2i+c, 32i+m]       = W0[m, c]        (K=8 rows used)
    #   l=1..8: wmat[32i+k, 128l+32i+m] = Wh[l-1][m, k]
    #   l=9:    wmat[32i+k, 1152+32i+m] = Wout[0, k]      (replicated x32)
    wmat = np.zeros((128, 1280), dtype=np.float32)
    wbias = np.zeros((128, 9), dtype=np.float32)
    for i in range(4):
        wmat[2 * i:2 * i + 2, 32 * i:32 * i + 32] = W0.T
        for hl in range(L):
            wmat[32 * i:32 * i + 32,
                 128 * (hl + 1) + 32 * i:128 * (hl + 1) + 32 * i + 32] = \
                Wh[hl].T
        wmat[32 * i:32 * i + 32, 1152 + 32 * i:1152 + 32 * i + 32] = \
            Wout[0, :, None]
        wbias[32 * i:32 * i + 32, 0] = b0
        for hl in range(L):
            wbias[32 * i:32 * i + 32, 1 + hl] = bh[hl]
    return wmat, wbias


def kernel(x, W0, b0, Wh, bh, Wout, bout):
    from concourse import bass_utils

    if "nc" not in _CACHE:
        _CACHE["nc"] = _build_nc()
    nc = _CACHE["nc"]

    wmat, wbias = _pack_weights(np.asarray(W0, np.float32),
                                np.asarray(b0, np.float32),
                                np.asarray(Wh, np.float32),
                                np.asarray(bh, np.float32),
                                np.asarray(Wout, np.float32))
    x = np.asarray(x, np.float32)
    in_maps = [_prep_core_inputs(x[c * R:(c + 1) * R], wmat, wbias)
               for c in range(N_CORES)]

    res = bass_utils.run_bass_kernel_spmd(nc, in_maps, list(range(N_CORES)))
    out = np.concatenate([r["out"].reshape(R) for r in res.results])
    return (out.reshape(N, 1) + np.float32(bout[0])).astype(np.float32)


# revision 22
# speedup vs baseline: 1.8256x; 1.4391x over previous
"""Trainium2 Bass kernel for a small MLP: [N,2] -> 32 -> (8x 32) -> 1.

Strategy (data-parallel over 8 cores, batch-sharded):
  - Per core R=262144 rows, processed in 32 supertiles of 8192 rows.
  - A supertile lives in SBUF as [128 partitions, 2048 free]: 4 partition
    blocks (32 hidden channels each) x 4 free blocks (512 rows each) = 16
    groups of 512 batch rows. Group (i,f) = rows s*8192+(4i+f)*512+[0,512).
  - Each layer = 4 matmuls of [K,128]x[K,512] with BLOCK-DIAGONAL weights:
    one instruction advances 4 groups (2048 batch rows) in 512 moving rows.
    float32r end-to-end (DRAM/SBUF tiles typed f32r; drains round to f32r)
    for the 1 cycle/row PE fast path.
  - bias+ReLU drain PSUM->SBUF split across ACT / DVE / Pool by rate.
  - Output layer: Wout replicated x32 -> every PSUM partition holds y;
    copy-drain, DMA partitions {0,32,64,96} to DRAM; bout added on host.
"""

import numpy as np

N = 2097152
H = 32
L = 8
N_CORES = 8
R = N // N_CORES          # 262144 rows per core
FB = 512                  # rows per group
ST_ROWS = 16 * FB         # 8192 rows per supertile
N_ST = R // ST_ROWS       # 32 supertiles per core

# Drain split (free columns of the 2048-wide supertile per engine),
# proportional to engine rates ACT 1.2 / DVE 0.96 cols/ns. GPSIMD (Pool)
# cannot access PSUM on TRN2, so drains are ACT+DVE only.
ACT_COLS = 1136
DVE_COLS = 2048 - ACT_COLS

_CACHE = {}


def _build_nc(n_st=N_ST):
    import concourse.tile as tile
    from concourse import bacc, mybir

    f32 = mybir.dt.float32
    f32r = mybir.dt.float32r

    nc = bacc.Bacc(None, target_bir_lowering=False)
    xt_d = nc.dram_tensor("xt", [8, n_st, 2048], f32r, kind="ExternalInput")
    wm_d = nc.dram_tensor("wmat", [128, 1280], f32r, kind="ExternalInput")
    wb_d = nc.dram_tensor("wbias", [128, 9], f32, kind="ExternalInput")
    out_d = nc.dram_tensor("out", [n_st, 4, 2048], f32, kind="ExternalOutput")

    relu = mybir.ActivationFunctionType.Relu
    alu_add = mybir.AluOpType.add
    alu_max = mybir.AluOpType.max

    with tile.TileContext(nc) as tc:
        with tc.tile_pool(name="wpool", bufs=1) as wpool, \
             tc.tile_pool(name="xpool", bufs=4) as xpool, \
             tc.tile_pool(name="hpool", bufs=4) as hpool, \
             tc.tile_pool(name="pspool", bufs=2, space="PSUM") as pspool:
            w = wpool.tile([128, 1280], f32r)
            nc.sync.dma_start(out=w[:], in_=wm_d[:, :])
            wb = wpool.tile([128, 9], f32)
            nc.sync.dma_start(out=wb[:], in_=wb_d[:, :])

            # Each group-layer uses TWO psum tiles and TWO SBUF out tiles:
            # psL (f-blocks 0,1) drained by ACT into hn01, psR (f-blocks
            # 2,3) drained by DVE into hn23. Sharing one psum/SBUF tile
            # serializes the two drains (the framework chains accesses to a
            # tile): mm->ACT(1.2us)->DVE(1.2us) was the critical chain
            # stalling the PE ~660ns per group. Separate tiles let ACT
            # start right after f1 and run concurrently with DVE.
            def layer(s, l, h01, h23, kdim):
                psL = pspool.tile([128, 1024], f32)
                psR = pspool.tile([128, 1024], f32)
                wcol = w[0:kdim, 128 * l:128 * (l + 1)]
                nc.tensor.matmul(psL[:, 0:512], wcol, h01[0:kdim, 0:512])
                nc.tensor.matmul(psL[:, 512:1024], wcol,
                                 h01[0:kdim, 512:1024])
                if l == 9:
                    ho01 = hpool.tile([128, 1024], f32)
                    nc.scalar.copy(ho01[:], psL[:])
                    nc.tensor.matmul(psR[:, 0:512], wcol, h23[0:kdim, 0:512])
                    nc.tensor.matmul(psR[:, 512:1024], wcol,
                                     h23[0:kdim, 512:1024])
                    ho23 = hpool.tile([128, 1024], f32)
                    nc.vector.tensor_scalar_add(ho23[:], psR[:], 0.0)
                    s01 = ho01[:].rearrange("(i r) f -> i r f", r=32)[:, 0, :]
                    nc.sync.dma_start(out=out_d[s, :, 0:1024], in_=s01)
                    s23 = ho23[:].rearrange("(i r) f -> i r f", r=32)[:, 0, :]
                    nc.sync.dma_start(out=out_d[s, :, 1024:2048], in_=s23)
                    return None, None
                bias = wb[:, l:l + 1]
                hn01 = hpool.tile([128, 1024], f32r)
                nc.scalar.activation(hn01[:], psL[:], relu, bias=bias)
                nc.tensor.matmul(psR[:, 0:512], wcol, h23[0:kdim, 0:512])
                nc.tensor.matmul(psR[:, 512:1024], wcol,
                                 h23[0:kdim, 512:1024])
                hn23 = hpool.tile([128, 1024], f32r)
                nc.vector.tensor_scalar(hn23[:], psR[:],
                                        bias, 0.0, alu_add, alu_max)
                return hn01, hn23

            # Four supertiles in flight: while supertile A's layer-l drain
            # runs on ACT/DVE, the PE does B/C/D's layer-l matmuls. Removes
            # per-layer PE stalls and keeps the PE continuously busy
            # (p-state ramp needs 3us of uninterrupted execution).
            for t in range(n_st // 4):
                ss = [4 * t + j for j in range(4)]
                hs = []
                for s in ss:
                    x01 = xpool.tile([8, 1024], f32r)
                    nc.sync.dma_start(out=x01[:], in_=xt_d[:, s, 0:1024])
                    x23 = xpool.tile([8, 1024], f32r)
                    nc.sync.dma_start(out=x23[:], in_=xt_d[:, s, 1024:2048])
                    hs.append((x01, x23))
                kdim = 8
                for l in range(10):
                    hs = [layer(ss[j], l, hs[j][0], hs[j][1], kdim)
                          for j in range(4)]
                    kdim = 128
    nc.finalize()
    return nc


def _prep_core_inputs(x_shard, wmat, wbias):
    # xt[2i+c, s, 512f+r] = x_shard[s*8192 + (4i+f)*512 + r, c]
    xs = np.ascontiguousarray(x_shard, dtype=np.float32).reshape(
        N_ST, 4, 4, FB, 2)
    xt = np.ascontiguousarray(xs.transpose(1, 4, 0, 2, 3)).reshape(
        8, N_ST, 2048)
    return {"xt": xt, "wmat": wmat, "wbias": wbias}


def _pack_weights(W0, b0, Wh, bh, Wout):
    # Block-diagonal lhsT per layer, 128 cols each:
    #   l=0:    wmat[2i+c, 32i+m]       = W0[m, c]        (K=8 rows used)
    #   l=1..8: wmat[32i+k, 128l+32i+m] = Wh[l-1][m, k]
    #   l=9:    wmat[32i+k, 1152+32i+m] = Wout[0, k]      (replicated x32)
    wmat = np.zeros((128, 1280), dtype=np.float32)
    wbias = np.zeros((128, 9), dtype=np.float32)
    for i in range(4):
        wmat[2 * i:2 * i + 2, 32 * i:32 * i + 32] = W0.T
        for hl in range(L):
            wmat[32 * i:32 * i + 32,
                 128 * (hl + 1) + 32 * i:128 * (hl + 1) + 32 * i + 32] = \
                Wh[hl].T
        wmat[32 * i:32 * i + 32, 1152 + 32 * i:1152 + 32 * i + 32] = \
            Wout[0, :, None]
        wbias[32 * i:32 * i + 32, 0] = b0
        for hl in range(L):
            wbias[32 * i:32 * i + 32, 1 + hl] = bh[hl]
    return wmat, wbias


def kernel(x, W0, b0, Wh, bh, Wout, bout):
    from concourse import bass_utils

    if "nc" not in _CACHE:
        _CACHE["nc"] = _build_nc()
    nc = _CACHE["nc"]

    wmat, wbias = _pack_weights(np.asarray(W0, np.float32),
                                np.asarray(b0, np.float32),
                                np.asarray(Wh, np.float32),
                                np.asarray(bh, np.float32),
                                np.asarray(Wout, np.float32))
    x = np.asarray(x, np.float32)
    in_maps = [_prep_core_inputs(x[c * R:(c + 1) * R], wmat, wbias)
               for c in range(N_CORES)]

    res = bass_utils.run_bass_kernel_spmd(nc, in_maps, list(range(N_CORES)))
    out = np.concatenate([r["out"].reshape(R) for r in res.results])
    return (out.reshape(N, 1) + np.float32(bout[0])).astype(np.float32)
